# revision 1
# baseline (speedup 1.0000x reference)
import os
import sys
import types
from contextlib import ExitStack

sys.path.insert(0, "/opt/trn_rl_repo")

import numpy as np

import concourse.bacc as bacc
import concourse.tile as tile
import concourse.mybir as mybir
from concourse import bass_utils
from concourse.bass_utils import run_bass_kernel_spmd

NCORES = 8
B, N, HX, HS = 32, 4096, 128, 1024
F = 512            # HX * R
COLS = 16384       # W columns per core
NB = 32            # 512-col param blocks per core
SPC = B // NCORES  # samples per core
TS = 512           # tokens per block
TB = N // TS

USE_FP32R = os.environ.get("KERNEL_FP32R", "0") == "1"

LAST_EXEC_NS = None
_cached_nc = None


def _ensure_axon_hooks():
    try:
        import antenv.axon_hooks  # noqa: F401
        return
    except Exception:
        pass
    hook = None
    try:
        import trn_agent_boot.trn_boot as tb
        hook = tb._ntff_profile_via_ctypes("/opt/axon/libaxon_pjrt.so")
    except Exception:
        hook = None
    mod = types.ModuleType("antenv.axon_hooks")
    mod.get_axon_ntff_profile_hook = lambda: hook
    sys.modules["antenv.axon_hooks"] = mod
    try:
        bass_utils.upload_artifacts = lambda tmpdir: tmpdir
    except Exception:
        pass


def _build():
    fp32 = mybir.dt.float32
    AF = mybir.ActivationFunctionType
    ALU = mybir.AluOpType

    def mm(ap):
        return ap.bitcast(mybir.dt.float32r) if USE_FP32R else ap

    nc = bacc.Bacc("TRN2", target_bir_lowering=False, debug=False,
                   num_devices=NCORES)
    W_d = nc.dram_tensor("W", [NB, 128, 8 * F], fp32, kind="ExternalInput")
    x_d = nc.dram_tensor("x", [SPC, HX, N], fp32, kind="ExternalInput")
    s_d = nc.dram_tensor("s", [128, 8 * B], fp32, kind="ExternalInput")
    b_d = nc.dram_tensor("b", [NB, B, F], fp32, kind="ExternalInput")
    g_d = nc.dram_tensor("g", [HX, 1], fp32, kind="ExternalInput")
    o_d = nc.dram_tensor("o", [SPC, HX, N], fp32, kind="ExternalOutput")

    with tile.TileContext(nc) as tc:
        with tc.tile_pool(name="pers", bufs=1) as pers, \
             tc.tile_pool(name="xres", bufs=1) as xres, \
             tc.tile_pool(name="dram", bufs=1, space="DRAM") as dram:
            s_t = pers.tile([128, 8 * B], fp32)
            nc.sync.dma_start(s_t[:], s_d[:])
            g_t = pers.tile([HX, 1], fp32)
            nc.sync.dma_start(g_t[:], g_d[:])
            ones_col = pers.tile([128, 1], fp32)
            nc.vector.memset(ones_col[:], 1.0)
            ones_row = pers.tile([1, 128], fp32)
            nc.vector.memset(ones_row[:], 1.0)
            eps_t = pers.tile([128, 1], fp32)
            nc.vector.memset(eps_t[:], 1e-6)

            xts = []
            for i in range(SPC):
                xt = xres.tile([HX, N], fp32)
                nc.sync.dma_start(xt[:], x_d[i, :, :])
                xts.append(xt)

            in_b = dram.tile([B, COLS], fp32)
            out_b = dram.tile([B, COLS], fp32)

            # phase A: params = s @ W + b for this core's 16384 columns
            with tc.tile_pool(name="wp", bufs=2) as wp, \
                 tc.tile_pool(name="bt", bufs=2) as btp, \
                 tc.tile_pool(name="stg", bufs=2) as stg, \
                 tc.tile_pool(name="psA", bufs=2, space="PSUM") as psA:
                for nb in range(NB):
                    wt = wp.tile([128, 8 * F], fp32)
                    nc.sync.dma_start(wt[:], W_d[nb, :, :])
                    bt = btp.tile([B, F], fp32)
                    nc.sync.dma_start(bt[:], b_d[nb, :, :])
                    ps = psA.tile([B, F], fp32)
                    for kt in range(8):
                        nc.tensor.matmul(
                            ps[:],
                            mm(s_t[:, kt * B:(kt + 1) * B]),
                            mm(wt[:, kt * F:(kt + 1) * F]),
                            start=(kt == 0), stop=(kt == 7),
                        )
                    st = stg.tile([B, F], fp32)
                    nc.vector.tensor_tensor(st[:], ps[:], bt[:], ALU.add)
                    nc.sync.dma_start(in_b[:, nb * F:(nb + 1) * F], st[:])

            # all-to-all: row 4*src+i on this core <- core src's params for
            # this core's local sample i
            nc.gpsimd.collective_compute(
                "AllToAll", ALU.bypass,
                replica_groups=[list(range(NCORES))],
                ins=[in_b.opt()], outs=[out_b.opt()],
            )

            # phase C: per-sample weight norms + rmsnorm + bmm1/silu/bmm2
            with ExitStack() as es:
                def pool(name, bufs, space=None):
                    kw = {"space": space} if space else {}
                    return es.enter_context(
                        tc.tile_pool(name=name, bufs=bufs, **kw))
                p_fc1 = pool("fc1", 2)
                p_fc1g = pool("fc1g", 2)
                p_fc2 = pool("fc2", 8)
                p_sq = pool("sq", 2)
                p_rn = pool("rn", 4)
                p_tmp = pool("tmp", 4)
                p_xsq = pool("xsq", 2)
                p_sqm = pool("sqm", 2)
                p_rr = pool("rr", 2)
                p_xs = pool("xs", 2)
                p_s1 = pool("s1", 2)
                p_h1 = pool("h1", 2)
                p_ob = pool("ob", 2)
                p_pn = pool("pn", 2, "PSUM")
                p_pss = pool("pss", 1, "PSUM")
                p_psb = pool("psb", 1, "PSUM")
                p_ph1 = pool("ph1", 2, "PSUM")
                p_ph2 = pool("ph2", 2, "PSUM")
                for i in range(SPC):
                    fc1r = p_fc1.tile([HX, F], fp32)
                    for src in range(4):
                        r = 4 * src + i
                        nc.sync.dma_start(
                            fc1r[32 * src:32 * (src + 1), :],
                            out_b[r:r + 1, :].rearrange(
                                "o (a f) -> (o a) f", a=32),
                        )
                    fc2r = []
                    for fb in range(4):
                        r = 16 + 4 * fb + i
                        t = p_fc2.tile([128, HX], fp32)
                        nc.sync.dma_start(
                            t[:],
                            out_b[r:r + 1, :].rearrange(
                                "o (p q) -> (o p) q", p=128),
                        )
                        fc2r.append(t)

                    # fc1 col norms over d -> rn1 [f_part, fb]
                    sq1 = p_sq.tile([HX, F], fp32)
                    nc.vector.tensor_tensor(sq1[:], fc1r[:], fc1r[:], ALU.mult)
                    pn1 = p_pn.tile([128, 4], fp32, name="pn")
                    for fb in range(4):
                        nc.tensor.matmul(
                            pn1[:, fb:fb + 1],
                            mm(sq1[:, fb * 128:(fb + 1) * 128]),
                            mm(ones_col[:]),
                            start=True, stop=True,
                        )
                    n1 = p_tmp.tile([128, 4], fp32)
                    nc.scalar.activation(n1[:], pn1[:], AF.Sqrt)
                    n1m = p_tmp.tile([128, 4], fp32)
                    nc.vector.tensor_scalar_max(n1m[:], n1[:], 1e-12)
                    rn1 = p_rn.tile([128, 4], fp32)
                    nc.vector.reciprocal(rn1[:], n1m[:])

                    # fc2 col norms over f -> rn2 [d_part, 1]
                    sq2 = p_sq.tile([128, F], fp32)
                    for fb in range(4):
                        nc.vector.tensor_tensor(
                            sq2[:, fb * 128:(fb + 1) * 128],
                            fc2r[fb][:], fc2r[fb][:], ALU.mult)
                    pn2 = p_pn.tile([128, 4], fp32, name="pn")
                    for fb in range(4):
                        nc.tensor.matmul(
                            pn2[:, 0:1],
                            mm(sq2[:, fb * 128:(fb + 1) * 128]),
                            mm(ones_col[:]),
                            start=(fb == 0), stop=(fb == 3),
                        )
                    n2 = p_tmp.tile([128, 1], fp32)
                    nc.scalar.activation(n2[:], pn2[:, 0:1], AF.Sqrt)
                    n2m = p_tmp.tile([128, 1], fp32)
                    nc.vector.tensor_scalar_max(n2m[:], n2[:], 1e-12)
                    rn2 = p_rn.tile([128, 1], fp32)
                    nc.vector.reciprocal(rn2[:], n2m[:])

                    # fold rmsnorm weight g into fc1 rows (per-partition d)
                    fc1g = p_fc1g.tile([HX, F], fp32)
                    nc.vector.tensor_scalar_mul(fc1g[:], fc1r[:], g_t[:])

                    xt = xts[i]
                    for tb in range(TB):
                        xv = xt[:, tb * TS:(tb + 1) * TS]
                        xsq = p_xsq.tile([HX, TS], fp32)
                        nc.gpsimd.tensor_tensor(xsq[:], xv, xv, ALU.mult)
                        pss = p_pss.tile([1, TS], fp32)
                        nc.tensor.matmul(pss[:], mm(ones_col[:]), mm(xsq[:]),
                                         start=True, stop=True)
                        s1 = p_s1.tile([1, TS], fp32)
                        nc.scalar.activation(s1[:], pss[:], AF.Copy)
                        psb = p_psb.tile([HX, TS], fp32)
                        nc.tensor.matmul(psb[:], mm(ones_row[:]), mm(s1[:]),
                                         start=True, stop=True)
                        sqm = p_sqm.tile([HX, TS], fp32)
                        nc.scalar.activation(sqm[:], psb[:], AF.Sqrt,
                                             bias=eps_t[:], scale=1.0 / HX)
                        rr = p_rr.tile([HX, TS], fp32)
                        nc.vector.reciprocal(rr[:], sqm[:])
                        xs = p_xs.tile([HX, TS], fp32)
                        nc.vector.tensor_tensor(xs[:], xv, rr[:], ALU.mult)

                        ph2 = p_ph2.tile([HX, TS], fp32)
                        for fb in range(4):
                            ph1 = p_ph1.tile([128, TS], fp32)
                            nc.tensor.matmul(
                                ph1[:],
                                mm(fc1g[:, fb * 128:(fb + 1) * 128]),
                                mm(xs[:]),
                                start=True, stop=True,
                            )
                            h1 = p_h1.tile([128, TS], fp32)
                            nc.scalar.activation(h1[:], ph1[:], AF.Silu,
                                                 scale=rn1[:, fb:fb + 1])
                            nc.tensor.matmul(
                                ph2[:], mm(fc2r[fb][:]), mm(h1[:]),
                                start=(fb == 0), stop=(fb == 3),
                            )
                        ob = p_ob.tile([HX, TS], fp32)
                        nc.vector.scalar_tensor_tensor(
                            ob[:], ph2[:], rn2[:], xv, ALU.mult, ALU.add)
                        nc.sync.dma_start(o_d[i, :, tb * TS:(tb + 1) * TS],
                                          ob[:])
    nc.compile()
    return nc


def _prep_inputs(x, s, W, b, g):
    s_p = np.ascontiguousarray(
        s.T.reshape(8, 128, B).transpose(1, 0, 2).reshape(128, 8 * B))
    g_p = np.ascontiguousarray(g.reshape(HX, 1))
    in_maps = []
    for c in range(NCORES):
        Wc = W[:, c * COLS:(c + 1) * COLS]
        Wc = np.ascontiguousarray(
            Wc.reshape(8, 128, NB, F).transpose(2, 1, 0, 3)
              .reshape(NB, 128, 8 * F))
        bc = np.ascontiguousarray(np.broadcast_to(
            b[c * COLS:(c + 1) * COLS].reshape(NB, 1, F), (NB, B, F)))
        xc = np.ascontiguousarray(
            x[SPC * c:SPC * (c + 1)].transpose(0, 2, 1))
        in_maps.append({"W": Wc, "x": xc, "s": s_p, "b": bc, "g": g_p})
    return in_maps


def kernel(x, s, W, b, g):
    global LAST_EXEC_NS, _cached_nc
    x = np.asarray(x, dtype=np.float32)
    s = np.asarray(s, dtype=np.float32)
    W = np.asarray(W, dtype=np.float32)
    b = np.asarray(b, dtype=np.float32)
    g = np.asarray(g, dtype=np.float32)

    trace = os.environ.get("KERNEL_TRACE", "0") == "1"
    if trace:
        _ensure_axon_hooks()
    if _cached_nc is None:
        _cached_nc = _build()
    in_maps = _prep_inputs(x, s, W, b, g)
    res = run_bass_kernel_spmd(_cached_nc, in_maps, list(range(NCORES)),
                               trace=trace)
    LAST_EXEC_NS = res.exec_time_ns
    out = np.concatenate([res.results[c]["o"] for c in range(NCORES)], axis=0)
    return np.ascontiguousarray(out.transpose(0, 2, 1))



# revision 13
# speedup vs baseline: 2.5773x; 2.5773x over previous
import os
import sys
import types
from contextlib import ExitStack

sys.path.insert(0, "/opt/trn_rl_repo")

import numpy as np
from ml_dtypes import bfloat16 as np_bf16

import concourse.bacc as bacc
import concourse.tile as tile
import concourse.mybir as mybir
from concourse import bass_utils, masks
from concourse.bass_utils import run_bass_kernel_spmd

NCORES = 8
B, N, HX, HS = 32, 4096, 128, 1024
F = 512            # HX * R
COLS = 16384       # W columns per core
NB = 32            # 512-col param blocks per core
NQ = 4             # collective stages (quarters of the nb loop)
NBQ = NB // NQ     # nb blocks per stage
SPC = B // NCORES  # samples per core
TS = 512           # tokens per block
TB = N // TS

LAST_EXEC_NS = None
_cached_nc = None


def _ensure_axon_hooks():
    try:
        import antenv.axon_hooks  # noqa: F401
        return
    except Exception:
        pass
    hook = None
    try:
        import trn_agent_boot.trn_boot as tb
        hook = tb._ntff_profile_via_ctypes("/opt/axon/libaxon_pjrt.so")
    except Exception:
        hook = None
    mod = types.ModuleType("antenv.axon_hooks")
    mod.get_axon_ntff_profile_hook = lambda: hook
    sys.modules["antenv.axon_hooks"] = mod
    try:
        bass_utils.upload_artifacts = lambda tmpdir: tmpdir
    except Exception:
        pass


def _build():
    fp32 = mybir.dt.float32
    bf16 = mybir.dt.bfloat16
    AF = mybir.ActivationFunctionType
    ALU = mybir.AluOpType

    nc = bacc.Bacc("TRN2", target_bir_lowering=False, debug=False,
                   num_devices=NCORES)
    W_d = nc.dram_tensor("W", [NB, 128, 8 * F], bf16, kind="ExternalInput")
    x_d = nc.dram_tensor("x", [SPC, HX, N], bf16, kind="ExternalInput")
    s_d = nc.dram_tensor("s", [128, 8 * B], bf16, kind="ExternalInput")
    b_d = nc.dram_tensor("b", [NB, B, F], bf16, kind="ExternalInput")
    g_d = nc.dram_tensor("g", [HX, 1], fp32, kind="ExternalInput")
    o_d = nc.dram_tensor("o", [SPC, HX, N], bf16, kind="ExternalOutput")

    with tile.TileContext(nc) as tc:
        with tc.tile_pool(name="pers", bufs=1) as pers, \
             tc.tile_pool(name="xres", bufs=1) as xres, \
             tc.tile_pool(name="dram", bufs=1, space="DRAM") as dram:
            s_t = pers.tile([128, 8 * B], bf16)
            nc.sync.dma_start(s_t[:], s_d[:])
            g_t = pers.tile([HX, 1], fp32)
            nc.sync.dma_start(g_t[:], g_d[:])
            ones_col = pers.tile([128, 1], bf16)
            nc.vector.memset(ones_col[:], 1.0)
            ones_row = pers.tile([1, 128], bf16)
            nc.vector.memset(ones_row[:], 1.0)
            eps_t = pers.tile([128, 1], fp32)
            nc.vector.memset(eps_t[:], 1e-6)
            ident = pers.tile([128, 128], fp32)
            masks.make_identity(nc, ident[:])

            # x loads stream on the Activation DGE queue alongside phase A's
            # W traffic (which owns the SP queue).
            xts = []
            for i in range(SPC):
                xt = xres.tile([HX, N], bf16, name=f"xt{i}")
                nc.scalar.dma_start(xt[:], x_d[i, :, :])
                xts.append(xt)

            in_all = dram.tile([NQ, B, NBQ * F], bf16, name="in_all")
            out_all = dram.tile([NQ, B, NBQ * F], bf16, name="out_all")
            in_bs = [in_all[q] for q in range(NQ)]
            out_bs = [out_all[q] for q in range(NQ)]

            rr_list = []
            # phase A: params = s @ W + b for this core's 16384 columns, in
            # NQ stages; each stage's all-to-all overlaps the next stage's
            # compute. rmsnorm stats for sample q are emitted in stage q's
            # collective shadow.
            with tc.tile_pool(name="wp", bufs=3) as wp, \
                 tc.tile_pool(name="bt", bufs=2) as btp, \
                 tc.tile_pool(name="stg", bufs=2) as stg, \
                 tc.tile_pool(name="xsqp", bufs=2) as xsqp, \
                 tc.tile_pool(name="stm", bufs=2) as stm, \
                 tc.tile_pool(name="psA", bufs=2, space="PSUM") as psA, \
                 tc.tile_pool(name="psS", bufs=2, space="PSUM") as psS:
                for q in range(NQ):
                    for nbl in range(NBQ):
                        nb = q * NBQ + nbl
                        wt = wp.tile([128, 8 * F], bf16)
                        nc.sync.dma_start(wt[:], W_d[nb, :, :])
                        bt = btp.tile([B, F], bf16)
                        nc.sync.dma_start(bt[:], b_d[nb, :, :])
                        ps = psA.tile([B, F], fp32)
                        for kt in range(8):
                            nc.tensor.matmul(
                                ps[:],
                                s_t[:, kt * B:(kt + 1) * B],
                                wt[:, kt * F:(kt + 1) * F],
                                start=(kt == 0), stop=(kt == 7),
                            )
                        st = stg.tile([B, F], bf16)
                        nc.vector.tensor_tensor(st[:], ps[:], bt[:], ALU.add)
                        nc.gpsimd.dma_start(
                            in_bs[q][:, nbl * F:(nbl + 1) * F], st[:])
                    nc.gpsimd.collective_compute(
                        "AllToAll", ALU.bypass,
                        replica_groups=[list(range(NCORES))],
                        ins=[in_bs[q].opt()], outs=[out_bs[q].opt()],
                    )

                    # rmsnorm stats for sample i=q -> rr_ts [32, 128] bf16,
                    # row gr, col p = 1/rms of token gr*128+p.
                    i = q
                    xt = xts[i]
                    xsq = xsqp.tile([HX, N], bf16)
                    nc.vector.tensor_tensor(xsq[:], xt[:], xt[:], ALU.mult)
                    pn_s = psS.tile([128, 32], fp32, name="pn_s")
                    for c in range(32):
                        nc.tensor.matmul(
                            pn_s[:, c:c + 1],
                            xsq[:, c * 128:(c + 1) * 128],
                            ones_col[:],
                            start=True, stop=True,
                        )
                    sq_m = stm.tile([128, 32], fp32, name="sq_m")
                    nc.scalar.activation(sq_m[:], pn_s[:], AF.Sqrt,
                                         bias=eps_t[:], scale=1.0 / HX)
                    rr = stm.tile([128, 32], fp32, name="rr")
                    nc.vector.reciprocal(rr[:], sq_m[:])
                    rr_t = psS.tile([32, 128], fp32, name="rr_t")
                    nc.tensor.transpose(rr_t[:], rr[:], ident[:])
                    rr_ts = stm.tile([32, 128], bf16, name="rr_ts")
                    nc.scalar.activation(rr_ts[:], rr_t[:], AF.Copy)
                    # flatten token-major onto one partition so the block
                    # loop can broadcast [1, TS] rows with base partition 0
                    rr_flat = pers.tile([1, N], bf16, name=f"rr_flat{i}")
                    nc.sync.dma_start(rr_flat[:], rr_ts[:])
                    rr_list.append(rr_flat)

            # phase C: per-sample weight norms + rmsnorm + bmm1/silu/bmm2
            with ExitStack() as es:
                def pool(name, bufs, space=None):
                    kw = {"space": space} if space else {}
                    return es.enter_context(
                        tc.tile_pool(name=name, bufs=bufs, **kw))
                p_fc1 = pool("fc1", 2)
                p_fc1g = pool("fc1g", 2)
                p_fc2 = pool("fc2", 2)
                p_sq = pool("sq", 2)
                p_rn = pool("rn", 4)
                p_tmp = pool("tmp", 4)
                p_xs = pool("xs", 2)
                p_h1 = pool("h1", 3)
                p_ob = pool("ob", 2)
                p_pn = pool("pn", 2, "PSUM")
                p_rrb = pool("rrb", 2, "PSUM")
                p_ph1 = pool("ph1", 2, "PSUM")
                p_ph2 = pool("ph2", 2, "PSUM")
                for i in range(SPC):
                    # fc1 gather: one DMA per source core covers all NQ
                    # stages (stage q holds d_rel in [8q, 8q+8)).
                    fc1r = p_fc1.tile([HX, F], bf16)
                    for src in range(4):
                        nc.sync.dma_start(
                            fc1r[32 * src:32 * (src + 1), :],
                            out_all[:, 4 * src + i:4 * src + i + 1, :].rearrange(
                                "q o (a f) -> q (o a) f", a=32 // NQ),
                        )
                    # fc2 gather: fc2c[f_local, (fb, d)]; slice fb gives the
                    # bmm2 stationary [f 128, d 128].
                    fc2c = p_fc2.tile([128, 4 * HX], bf16)
                    for fb in range(4):
                        nc.sync.dma_start(
                            fc2c[:, fb * HX:(fb + 1) * HX],
                            out_all[:, 16 + 4 * fb + i:17 + 4 * fb + i, :].rearrange(
                                "q o (p d) -> q (o p) d", p=128 // NQ),
                        )
                    fc2r = [fc2c[:, fb * HX:(fb + 1) * HX] for fb in range(4)]

                    # fc1 col norms over d -> rn1 [f_part, fb]
                    sq1 = p_sq.tile([HX, F], bf16)
                    nc.gpsimd.tensor_tensor(sq1[:], fc1r[:], fc1r[:],
                                            ALU.mult)
                    pn1 = p_pn.tile([128, 4], fp32, name="pn")
                    for fb in range(4):
                        nc.tensor.matmul(
                            pn1[:, fb:fb + 1],
                            sq1[:, fb * 128:(fb + 1) * 128],
                            ones_col[:],
                            start=True, stop=True,
                        )
                    n1 = p_tmp.tile([128, 4], fp32)
                    nc.scalar.activation(n1[:], pn1[:], AF.Sqrt)
                    n1m = p_tmp.tile([128, 4], fp32)
                    nc.vector.tensor_scalar_max(n1m[:], n1[:], 1e-12)
                    rn1 = p_rn.tile([128, 4], fp32)
                    nc.vector.reciprocal(rn1[:], n1m[:])

                    # fc2 col norms over f -> rn2 [d_part, 1]
                    sq2 = p_sq.tile([128, F], bf16)
                    nc.gpsimd.tensor_tensor(sq2[:], fc2c[:], fc2c[:],
                                            ALU.mult)
                    pn2 = p_pn.tile([128, 4], fp32, name="pn")
                    for fb in range(4):
                        nc.tensor.matmul(
                            pn2[:, 0:1],
                            sq2[:, fb * 128:(fb + 1) * 128],
                            ones_col[:],
                            start=(fb == 0), stop=(fb == 3),
                        )
                    n2 = p_tmp.tile([128, 1], fp32)
                    nc.scalar.activation(n2[:], pn2[:, 0:1], AF.Sqrt)
                    n2m = p_tmp.tile([128, 1], fp32)
                    nc.vector.tensor_scalar_max(n2m[:], n2[:], 1e-12)
                    rn2 = p_rn.tile([128, 1], fp32)
                    nc.vector.reciprocal(rn2[:], n2m[:])

                    # fold rmsnorm weight g into fc1 rows (per-partition d)
                    fc1g = p_fc1g.tile([HX, F], bf16)
                    nc.vector.tensor_scalar_mul(fc1g[:], fc1r[:], g_t[:])

                    xt = xts[i]
                    rr_flat = rr_list[i]
                    for tb in range(TB):
                        xv = xt[:, tb * TS:(tb + 1) * TS]
                        # broadcast 1/rms to [d, t] via a ones-row matmul
                        rrb = p_rrb.tile([HX, TS], fp32)
                        nc.tensor.matmul(
                            rrb[:],
                            ones_row[:],
                            rr_flat[0:1, tb * TS:(tb + 1) * TS],
                            start=True, stop=True,
                        )
                        xs = p_xs.tile([HX, TS], bf16)
                        nc.vector.tensor_tensor(xs[:], xv, rrb[:], ALU.mult)

                        ph2 = p_ph2.tile([HX, TS], fp32)
                        for fb in range(4):
                            ph1 = p_ph1.tile([128, TS], fp32)
                            nc.tensor.matmul(
                                ph1[:],
                                fc1g[:, fb * 128:(fb + 1) * 128],
                                xs[:],
                                start=True, stop=True,
                            )
                            h1 = p_h1.tile([128, TS], bf16)
                            nc.scalar.activation(h1[:], ph1[:], AF.Silu,
                                                 scale=rn1[:, fb:fb + 1])
                            nc.tensor.matmul(
                                ph2[:], fc2r[fb], h1[:],
                                start=(fb == 0), stop=(fb == 3),
                            )
                        ob = p_ob.tile([HX, TS], bf16)
                        nc.vector.scalar_tensor_tensor(
                            ob[:], ph2[:], rn2[:], xv, ALU.mult, ALU.add)
                        nc.sync.dma_start(
                            o_d[i, :, tb * TS:(tb + 1) * TS], ob[:])
    nc.compile()
    return nc


def _prep_inputs(x, s, W, b, g):
    s_p = np.ascontiguousarray(
        s.T.reshape(8, 128, B).transpose(1, 0, 2).reshape(128, 8 * B)
    ).astype(np_bf16)
    g_p = np.ascontiguousarray(g.reshape(HX, 1)).astype(np.float32)
    Wb = W.astype(np_bf16)
    bb = b.astype(np_bf16)
    in_maps = []
    for c in range(NCORES):
        Wc = Wb[:, c * COLS:(c + 1) * COLS]
        Wc = np.ascontiguousarray(
            Wc.reshape(8, 128, NB, F).transpose(2, 1, 0, 3)
              .reshape(NB, 128, 8 * F))
        bc = np.ascontiguousarray(np.broadcast_to(
            bb[c * COLS:(c + 1) * COLS].reshape(NB, 1, F), (NB, B, F)))
        xc = np.ascontiguousarray(
            x[SPC * c:SPC * (c + 1)].transpose(0, 2, 1)).astype(np_bf16)
        in_maps.append({"W": Wc, "x": xc, "s": s_p, "b": bc, "g": g_p})
    return in_maps


def kernel(x, s, W, b, g):
    global LAST_EXEC_NS, _cached_nc
    x = np.asarray(x, dtype=np.float32)
    s = np.asarray(s, dtype=np.float32)
    W = np.asarray(W, dtype=np.float32)
    b = np.asarray(b, dtype=np.float32)
    g = np.asarray(g, dtype=np.float32)

    trace = os.environ.get("KERNEL_TRACE", "0") == "1"
    if trace:
        _ensure_axon_hooks()
    if _cached_nc is None:
        _cached_nc = _build()
    in_maps = _prep_inputs(x, s, W, b, g)
    res = run_bass_kernel_spmd(_cached_nc, in_maps, list(range(NCORES)),
                               trace=trace)
    LAST_EXEC_NS = res.exec_time_ns
    out = np.concatenate([res.results[c]["o"] for c in range(NCORES)], axis=0)
    return np.ascontiguousarray(
        out.transpose(0, 2, 1).astype(np.float32))


# revision 18
# speedup vs baseline: 3.1231x; 1.2118x over previous
import os
import sys
import types
from contextlib import ExitStack

sys.path.insert(0, "/opt/trn_rl_repo")

import numpy as np
from ml_dtypes import bfloat16 as np_bf16

import concourse.bacc as bacc
import concourse.tile as tile
import concourse.mybir as mybir
from concourse import bass_utils, masks
from concourse.bass_utils import run_bass_kernel_spmd

NCORES = 8
B, N, HX, HS = 32, 4096, 128, 1024
F = 512            # HX * R
COLS = 16384       # W columns per core
NB = 32            # 512-col param blocks per core
NQ = 4             # collective stages
NBQ = NB // NQ     # nb blocks per stage
SPC = B // NCORES  # samples per core
TS = 512           # tokens per block
TB = N // TS

LAST_EXEC_NS = None
_cached_nc = None


def _ensure_axon_hooks():
    try:
        import antenv.axon_hooks  # noqa: F401
        return
    except Exception:
        pass
    hook = None
    try:
        import trn_agent_boot.trn_boot as tb
        hook = tb._ntff_profile_via_ctypes("/opt/axon/libaxon_pjrt.so")
    except Exception:
        hook = None
    mod = types.ModuleType("antenv.axon_hooks")
    mod.get_axon_ntff_profile_hook = lambda: hook
    sys.modules["antenv.axon_hooks"] = mod
    try:
        bass_utils.upload_artifacts = lambda tmpdir: tmpdir
    except Exception:
        pass


def _build():
    fp32 = mybir.dt.float32
    bf16 = mybir.dt.bfloat16
    AF = mybir.ActivationFunctionType
    ALU = mybir.AluOpType

    nc = bacc.Bacc("TRN2", target_bir_lowering=False, debug=False,
                   num_devices=NCORES)
    W_d = nc.dram_tensor("W", [NB, 128, 8 * F], bf16, kind="ExternalInput")
    x_d = nc.dram_tensor("x", [SPC, HX, N], bf16, kind="ExternalInput")
    s_d = nc.dram_tensor("s", [128, 8 * B], bf16, kind="ExternalInput")
    b_d = nc.dram_tensor("b", [NB, B, F], bf16, kind="ExternalInput")
    g_d = nc.dram_tensor("g", [HX, 1], fp32, kind="ExternalInput")
    o_d = nc.dram_tensor("o", [SPC, HX, N], bf16, kind="ExternalOutput")

    with tile.TileContext(nc) as tc:
        with tc.tile_pool(name="pers", bufs=1) as pers, \
             tc.tile_pool(name="xres", bufs=1) as xres, \
             tc.tile_pool(name="dram", bufs=1, space="DRAM") as dram:
            s_t = pers.tile([128, 8 * B], bf16)
            nc.sync.dma_start(s_t[:], s_d[:])
            g_t = pers.tile([HX, 1], fp32)
            nc.sync.dma_start(g_t[:], g_d[:])
            ones_col = pers.tile([128, 1], bf16)
            nc.vector.memset(ones_col[:], 1.0)
            ones_row = pers.tile([1, 128], bf16)
            nc.vector.memset(ones_row[:], 1.0)
            eps_t = pers.tile([128, 1], fp32)
            nc.vector.memset(eps_t[:], 1e-6)
            ident = pers.tile([128, 128], fp32)
            masks.make_identity(nc, ident[:])

            in_all = dram.tile([NQ, B, NBQ * F], bf16, name="in_all")
            out_all = dram.tile([NQ, B, NBQ * F], bf16, name="out_all")

            # phase A: params = s @ W + b for this core's 16384 columns, in
            # stages; each stage's all-to-all overlaps the next stage's
            # compute.
            with tc.tile_pool(name="wp", bufs=3) as wp, \
                 tc.tile_pool(name="bt", bufs=2) as btp, \
                 tc.tile_pool(name="stg", bufs=2) as stg, \
                 tc.tile_pool(name="psA", bufs=2, space="PSUM") as psA:
                for q in range(NQ):
                    for nbl in range(NBQ):
                        nb = q * NBQ + nbl
                        wt = wp.tile([128, 8 * F], bf16)
                        nc.sync.dma_start(wt[:], W_d[nb, :, :])
                        bt = btp.tile([B, F], bf16)
                        nc.sync.dma_start(bt[:], b_d[nb, :, :])
                        ps = psA.tile([B, F], fp32)
                        for kt in range(8):
                            nc.tensor.matmul(
                                ps[:],
                                s_t[:, kt * B:(kt + 1) * B],
                                wt[:, kt * F:(kt + 1) * F],
                                start=(kt == 0), stop=(kt == 7),
                            )
                        st = stg.tile([B, F], bf16)
                        nc.vector.tensor_tensor(st[:], ps[:], bt[:], ALU.add)
                        nc.gpsimd.dma_start(
                            in_all[q, :, nbl * F:(nbl + 1) * F], st[:])
                    nc.gpsimd.collective_compute(
                        "AllToAll", ALU.bypass,
                        replica_groups=[list(range(NCORES))],
                        ins=[in_all[q].opt()], outs=[out_all[q].opt()],
                    )

            # x loads go on the SP DMA ring AFTER all W traffic so they do
            # not steal phase A bandwidth; they feed the rmsnorm stats that
            # run in the final collective's shadow.
            xts = []
            for i in range(SPC):
                xt = xres.tile([HX, N], bf16, name=f"xt{i}")
                nc.sync.dma_start(xt[:], x_d[i, :, :])
                xts.append(xt)

            # rmsnorm stats -> rr_flat[i] [1, N] bf16, entry t = 1/rms of
            # token t.
            rr_list = []
            with tc.tile_pool(name="xsqp", bufs=2) as xsqp, \
                 tc.tile_pool(name="stm", bufs=2) as stm, \
                 tc.tile_pool(name="psS", bufs=2, space="PSUM") as psS:
                for i in range(SPC):
                    xt = xts[i]
                    xsq = xsqp.tile([HX, N], bf16)
                    nc.vector.tensor_tensor(xsq[:], xt[:], xt[:], ALU.mult)
                    pn_s = psS.tile([128, 32], fp32, name="pn_s")
                    for c in range(32):
                        nc.tensor.matmul(
                            pn_s[:, c:c + 1],
                            xsq[:, c * 128:(c + 1) * 128],
                            ones_col[:],
                            start=True, stop=True,
                        )
                    sq_m = stm.tile([128, 32], fp32, name="sq_m")
                    nc.scalar.activation(sq_m[:], pn_s[:], AF.Sqrt,
                                         bias=eps_t[:], scale=1.0 / HX)
                    rr = stm.tile([128, 32], fp32, name="rr")
                    nc.vector.reciprocal(rr[:], sq_m[:])
                    rr_t = psS.tile([32, 128], fp32, name="rr_t")
                    nc.tensor.transpose(rr_t[:], rr[:], ident[:])
                    rr_ts = stm.tile([32, 128], bf16, name="rr_ts")
                    nc.scalar.activation(rr_ts[:], rr_t[:], AF.Copy)
                    # flatten token-major onto one partition so the block
                    # loop can broadcast [1, TS] rows with base partition 0
                    rr_flat = pers.tile([1, N], bf16, name=f"rr_flat{i}")
                    nc.sync.dma_start(rr_flat[:], rr_ts[:])
                    rr_list.append(rr_flat)

            # phase C: per-sample weight norms, then a software-pipelined
            # loop over (sample, token-block) units where bmm1 of unit u
            # overlaps bmm2 of unit u-1.
            with ExitStack() as es:
                def pool(name, bufs, space=None):
                    kw = {"space": space} if space else {}
                    return es.enter_context(
                        tc.tile_pool(name=name, bufs=bufs, **kw))
                p_fc1 = pool("fc1", 4)
                p_fc1g = pool("fc1g", 4)
                p_fc2 = pool("fc2", 4)
                p_sq = pool("sq", 2)
                p_rn = pool("rn", 8)
                p_tmp = pool("tmp", 8)
                p_xs = pool("xs", 2)
                p_h1 = pool("h1", 4)
                p_ob = pool("ob", 2)
                p_pn = pool("pn", 1, "PSUM")
                p_rnb = pool("rnb", 1, "PSUM")
                p_rrb = pool("rrb", 1, "PSUM")
                p_ph2 = pool("ph2", 1, "PSUM")
                p_ph1a = pool("ph1a", 1, "PSUM")
                p_ph1b = pool("ph1b", 1, "PSUM")

                fc1gs, fc2cs, rn2s = [], [], []
                for i in range(SPC):
                    fc1r = p_fc1.tile([HX, F], bf16, name=f"fc1r{i}")
                    for src in range(4):
                        r = 4 * src + i
                        nc.sync.dma_start(
                            fc1r[32 * src:32 * (src + 1), :],
                            out_all[:, r:r + 1, :].rearrange(
                                "q o (a f) -> q (o a) f", a=32 // NQ),
                        )
                    fc2c = p_fc2.tile([128, 4 * HX], bf16, name=f"fc2c{i}")
                    for fb in range(4):
                        r = 16 + 4 * fb + i
                        nc.sync.dma_start(
                            fc2c[:, fb * HX:(fb + 1) * HX],
                            out_all[:, r:r + 1, :].rearrange(
                                "q o (p d) -> q (o p) d", p=128 // NQ),
                        )

                    # fc1 col norms over d -> rn1 [f_part, fb]
                    sq1 = p_sq.tile([HX, F], bf16)
                    nc.gpsimd.tensor_tensor(sq1[:], fc1r[:], fc1r[:],
                                            ALU.mult)
                    pn1 = p_pn.tile([128, 4], fp32, name="pscr")
                    for fb in range(4):
                        nc.tensor.matmul(
                            pn1[:, fb:fb + 1],
                            sq1[:, fb * 128:(fb + 1) * 128],
                            ones_col[:],
                            start=True, stop=True,
                        )
                    n1 = p_tmp.tile([128, 4], fp32)
                    nc.scalar.activation(n1[:], pn1[:], AF.Sqrt)
                    n1m = p_tmp.tile([128, 4], fp32)
                    nc.vector.tensor_scalar_max(n1m[:], n1[:], 1e-12)
                    rn1 = p_rn.tile([128, 4], fp32)
                    nc.vector.reciprocal(rn1[:], n1m[:])
                    # transpose rn1 to a [1, F] row and broadcast over d so
                    # it can fold into fc1g (then silu needs no scale and
                    # can run on [128, 2*TS] tiles)
                    trn1 = p_pn.tile([4, 128], fp32, name="pscr")
                    nc.tensor.transpose(trn1[:], rn1[:], ident[:])
                    trn1s = p_tmp.tile([4, 128], bf16)
                    nc.scalar.activation(trn1s[:], trn1[:], AF.Copy)
                    rn1_flat = p_tmp.tile([1, F], bf16)
                    nc.sync.dma_start(rn1_flat[:], trn1s[:])
                    rn1b = p_rnb.tile([HX, F], fp32, name="rn1b")
                    nc.tensor.matmul(rn1b[:], ones_row[:], rn1_flat[:],
                                     start=True, stop=True)

                    # fc2 col norms over f -> rn2 [d_part, 1]
                    sq2 = p_sq.tile([128, F], bf16)
                    nc.gpsimd.tensor_tensor(sq2[:], fc2c[:], fc2c[:],
                                            ALU.mult)
                    pn2 = p_pn.tile([128, 4], fp32, name="pscr")
                    for fb in range(4):
                        nc.tensor.matmul(
                            pn2[:, 0:1],
                            sq2[:, fb * 128:(fb + 1) * 128],
                            ones_col[:],
                            start=(fb == 0), stop=(fb == 3),
                        )
                    n2 = p_tmp.tile([128, 1], fp32)
                    nc.scalar.activation(n2[:], pn2[:, 0:1], AF.Sqrt)
                    n2m = p_tmp.tile([128, 1], fp32)
                    nc.vector.tensor_scalar_max(n2m[:], n2[:], 1e-12)
                    rn2 = p_rn.tile([128, 1], fp32, name=f"rn2_{i}")
                    nc.vector.reciprocal(rn2[:], n2m[:])

                    # fc1g = fc1r * g (per-partition d) * rn1 (per-column f)
                    fc1g = p_fc1g.tile([HX, F], bf16, name=f"fc1g{i}")
                    nc.vector.scalar_tensor_tensor(
                        fc1g[:], fc1r[:], g_t[:], rn1b[:],
                        ALU.mult, ALU.mult)

                    fc1gs.append(fc1g)
                    fc2cs.append(fc2c)
                    rn2s.append(rn2)

                units = [(i, tb) for i in range(SPC) for tb in range(TB)]
                prev = None

                def bmm2_first(pv):
                    nc.tensor.matmul(pv["ph2"][:],
                                     fc2cs[pv["i"]][:, 0:HX],
                                     pv["h1a"][:, 0:TS],
                                     start=True, stop=False)
                    nc.tensor.matmul(pv["ph2"][:],
                                     fc2cs[pv["i"]][:, HX:2 * HX],
                                     pv["h1a"][:, TS:2 * TS],
                                     start=False, stop=False)

                def bmm2_second(pv):
                    nc.tensor.matmul(pv["ph2"][:],
                                     fc2cs[pv["i"]][:, 2 * HX:3 * HX],
                                     pv["h1b"][:, 0:TS],
                                     start=False, stop=False)
                    nc.tensor.matmul(pv["ph2"][:],
                                     fc2cs[pv["i"]][:, 3 * HX:4 * HX],
                                     pv["h1b"][:, TS:2 * TS],
                                     start=False, stop=True)
                    ob = p_ob.tile([HX, TS], bf16)
                    nc.vector.scalar_tensor_tensor(
                        ob[:], pv["ph2"][:], rn2s[pv["i"]][:], pv["xv"],
                        ALU.mult, ALU.add)
                    nc.sync.dma_start(
                        o_d[pv["i"], :, pv["tb"] * TS:(pv["tb"] + 1) * TS],
                        ob[:])

                for i, tb in units:
                    xt = xts[i]
                    xv = xt[:, tb * TS:(tb + 1) * TS]
                    rrb = p_rrb.tile([HX, TS], fp32, name="rrb")
                    nc.tensor.matmul(
                        rrb[:], ones_row[:],
                        rr_list[i][0:1, tb * TS:(tb + 1) * TS],
                        start=True, stop=True,
                    )
                    xs = p_xs.tile([HX, TS], bf16)
                    nc.vector.tensor_tensor(xs[:], xv, rrb[:], ALU.mult)

                    fc1g = fc1gs[i]
                    ph1a = p_ph1a.tile([128, 2 * TS], fp32)
                    nc.tensor.matmul(ph1a[:, 0:TS], fc1g[:, 0:128], xs[:],
                                     start=True, stop=True)
                    nc.tensor.matmul(ph1a[:, TS:2 * TS], fc1g[:, 128:256],
                                     xs[:], start=True, stop=True)
                    h1a = p_h1.tile([128, 2 * TS], bf16)
                    nc.scalar.activation(h1a[:], ph1a[:], AF.Silu)

                    if prev is not None:
                        prev["ph2"] = p_ph2.tile([HX, TS], fp32, name="ph2")
                        bmm2_first(prev)

                    ph1b = p_ph1b.tile([128, 2 * TS], fp32)
                    nc.tensor.matmul(ph1b[:, 0:TS], fc1g[:, 256:384], xs[:],
                                     start=True, stop=True)
                    nc.tensor.matmul(ph1b[:, TS:2 * TS], fc1g[:, 384:512],
                                     xs[:], start=True, stop=True)
                    h1b = p_h1.tile([128, 2 * TS], bf16)
                    nc.scalar.activation(h1b[:], ph1b[:], AF.Silu)

                    if prev is not None:
                        bmm2_second(prev)

                    prev = {"i": i, "tb": tb, "xv": xv,
                            "h1a": h1a, "h1b": h1b}

                prev["ph2"] = p_ph2.tile([HX, TS], fp32, name="ph2")
                bmm2_first(prev)
                bmm2_second(prev)
    nc.compile()
    return nc


def _prep_inputs(x, s, W, b, g):
    s_p = np.ascontiguousarray(
        s.T.reshape(8, 128, B).transpose(1, 0, 2).reshape(128, 8 * B)
    ).astype(np_bf16)
    g_p = np.ascontiguousarray(g.reshape(HX, 1)).astype(np.float32)
    Wb = W.astype(np_bf16)
    bb = b.astype(np_bf16)
    in_maps = []
    for c in range(NCORES):
        Wc = Wb[:, c * COLS:(c + 1) * COLS]
        Wc = np.ascontiguousarray(
            Wc.reshape(8, 128, NB, F).transpose(2, 1, 0, 3)
              .reshape(NB, 128, 8 * F))
        bc = np.ascontiguousarray(np.broadcast_to(
            bb[c * COLS:(c + 1) * COLS].reshape(NB, 1, F), (NB, B, F)))
        xc = np.ascontiguousarray(
            x[SPC * c:SPC * (c + 1)].transpose(0, 2, 1)).astype(np_bf16)
        in_maps.append({"W": Wc, "x": xc, "s": s_p, "b": bc, "g": g_p})
    return in_maps


def kernel(x, s, W, b, g):
    global LAST_EXEC_NS, _cached_nc
    x = np.asarray(x, dtype=np.float32)
    s = np.asarray(s, dtype=np.float32)
    W = np.asarray(W, dtype=np.float32)
    b = np.asarray(b, dtype=np.float32)
    g = np.asarray(g, dtype=np.float32)

    trace = os.environ.get("KERNEL_TRACE", "0") == "1"
    if trace:
        _ensure_axon_hooks()
    if _cached_nc is None:
        _cached_nc = _build()
    in_maps = _prep_inputs(x, s, W, b, g)
    res = run_bass_kernel_spmd(_cached_nc, in_maps, list(range(NCORES)),
                               trace=trace)
    LAST_EXEC_NS = res.exec_time_ns
    out = np.concatenate([res.results[c]["o"] for c in range(NCORES)], axis=0)
    return np.ascontiguousarray(
        out.transpose(0, 2, 1).astype(np.float32))
